# revision 18
# baseline (speedup 1.0000x reference)
"""Trainium2 Bass kernel for nn_Decoder (input proj -> relu RNN -> 2-layer head).

Strategy (8 NeuronCores, pure batch data-parallelism, B=32 rows/core):
  - Fold the input projection into the recurrence drive on the host:
        f_t = W_eff @ x_t^T,  W_eff = 64 * (W_rec @ W_in)  (fp8 e4m3, x in fp8)
        s_{t+1} = relu(W_rec s_t + f_t/64 + b_eff)
  - x is host-pretransposed to [p, n, k] (s = k*128+p, n = t*B+b) and
    host-cast to fp8e4, so NO on-chip transposes and HALF the HBM bytes of
    bf16.  All 16.8MB/core stay RESIDENT in SBUF.
  - F GEMM runs as fp8 DoubleRow matmuls (0.5 cyc/col, K=256 per
    instruction), accumulating into PSUM partitions 0:63; a gpsimd
    (Pool-engine) tensor_scalar evicts f = PSUM/64 + b_eff into the [f; s]
    state buffer (f on partitions 0:63, s on 64:127) as bf16.
  - The 512-step recurrence runs as 16 CONCURRENT chains of 32 real steps
    (+8 warm steps from zero; ||W_rec||~0.34 so 0.34^8 ~ 2e-4).  Chains are
    fused into 2 groups of 8: one [128Kx64M, 256N] matmul per group-step
    (lhsT = [[I/64],[W_rec^T]], tile_position (0,64)), relu on DVE for
    group A and on ScalarE for group B -> ~850ns per fused step for all
    16 chains.
  - x streams in 64 half-chunks (256 n-cols, 2KB runs) ordered by consume
    deadline: warm sources (= r3 blocks of the previous chain) first, then
    wave r0/r1/r2, chain-15's final half last.  GEMM emission is staged
    between step blocks so the in-order PE queue never deadlocks.
  - Head: 4 chunks' relu(W_o1 s + b_o1) stacked on partitions via
    tile_position (64, 32i) -> ONE [128,512] relu -> one [128K, 8M] W_o2
    matmul -> PSUM->SBUF copy -> DMA out.  b_o2 is added on the host.
"""

import sys
import json
import numpy as np

for _p in ("/opt/trn_rl_repo",):
    if _p not in sys.path:
        sys.path.insert(0, _p)

import ml_dtypes
import concourse.bass as bass
import concourse.mybir as mybir
import concourse.tile as tile
from concourse.bass_utils import run_bass_kernel_spmd
from contextlib import ExitStack

BS, T, S, H = 256, 512, 1024, 64
NCORES = 8
B = BS // NCORES          # 32 batch rows per core
N = T * B                 # 16384 columns (n = t*B + b)
G = 16                    # chains
GW = 8                    # chains per fused group
RSTEP = T // G            # 32 real steps per chain
WARM = 8
SPC = WARM + RSTEP        # 40 steps per chain
SEC = (SPC + 1) * B       # 1312 cols per chain section
WSCALE = 64.0

F32 = mybir.dt.float32
BF16 = mybir.dt.bfloat16
FP8 = mybir.dt.float8e4
DR = mybir.MatmulPerfMode.DoubleRow
RELU = mybir.ActivationFunctionType.Relu

# Build-time config (A/B testing): GEMM mode and x-DMA granularity.
import os as _os
CFG = {"gemm": _os.environ.get("GEMM_MODE", "ft"),
       "nh": int(_os.environ.get("DMA_NH", "2"))}


def _split_multiwaits(nc, max_waits=1):
    """walrus in this container rejects >1 sem-wait on one instruction (the
    Tile end-of-kernel drain carries several).  Split extras into chained
    same-engine NoOps, then pin the serialized bytes on the nc object."""
    j = json.loads(nc.to_json_bytes())
    for f in j["functions"]:
        for bb in f["blocks"]:
            newinsts = []
            for inst in bb["instructions"]:
                si = inst.get("sync_info")
                waits = (si or {}).get("on_wait") or []
                if len(waits) > max_waits:
                    for k, w in enumerate(waits[max_waits:]):
                        newinsts.append({
                            "debug": inst.get("debug"),
                            "engine": inst["engine"],
                            "ins": [], "outs": [],
                            "name": f'{inst["name"]}-xw{k}',
                            "opcode": "NoOp",
                            "sync_info": {"on_update": [], "on_wait": [w]},
                        })
                    si["on_wait"] = waits[:max_waits]
                newinsts.append(inst)
            bb["instructions"] = newinsts
    b = json.dumps(j).encode()
    nc.to_json_bytes = lambda: b
    return nc


def build_decoder_nc(repeats=1):
    nc = bass.Bass("TRN2", target_bir_lowering=False, debug=False)

    xt_d = nc.dram_tensor("xt", [128, N * 8], FP8, kind="ExternalInput")
    wpack_d = nc.dram_tensor("wpack", [128, 512], FP8, kind="ExternalInput")
    wi_d = nc.dram_tensor("wi", [128, H], BF16, kind="ExternalInput")
    wo1t_d = nc.dram_tensor("wo1t", [H, 32], BF16, kind="ExternalInput")
    wo2t4_d = nc.dram_tensor("wo2t4", [128, 8], BF16, kind="ExternalInput")
    beff_d = nc.dram_tensor("beff", [H, 1], F32, kind="ExternalInput")
    bo1r_d = nc.dram_tensor("bo1r", [128, 1], F32, kind="ExternalInput")
    out_d = nc.dram_tensor("out4", [8, 8 * 512], F32, kind="ExternalOutput")

    with tile.TileContext(nc) as tc:
        with ExitStack() as ctx:
            consts = ctx.enter_context(tc.tile_pool(name="consts", bufs=1))
            xpool = ctx.enter_context(tc.tile_pool(name="xt", bufs=1))
            spool = ctx.enter_context(tc.tile_pool(name="sf", bufs=1))
            hspool = ctx.enter_context(tc.tile_pool(name="hs", bufs=2))
            opool = ctx.enter_context(tc.tile_pool(name="osb", bufs=2))
            gemm_mode = CFG["gemm"]
            if gemm_mode == "ft":
                ftp_pool = ctx.enter_context(
                    tc.tile_pool(name="ftp", bufs=2, space="PSUM"))
                ftr_pool = ctx.enter_context(
                    tc.tile_pool(name="ftr", bufs=2, space="PSUM"))
            else:
                fps_pool = ctx.enter_context(
                    tc.tile_pool(name="fps", bufs=3, space="PSUM"))
            rAB_pool = ctx.enter_context(
                tc.tile_pool(name="rAB", bufs=1, space="PSUM"))
            hps_pool = ctx.enter_context(
                tc.tile_pool(name="hps", bufs=2, space="PSUM"))
            ops_pool = ctx.enter_context(
                tc.tile_pool(name="ops", bufs=1, space="PSUM"))

            # --- constants (loaded once); GEMM/step consts go first, the
            # head consts are issued behind the x stream (needed ~j24) ---
            wpack_sb = consts.tile([128, 512], FP8)
            nc.sync.dma_start(out=wpack_sb, in_=wpack_d.ap())
            wp4 = wpack_sb.rearrange("p (a d h) -> p a d h", a=4, d=2)
            wi_sb = consts.tile([128, H], BF16)
            nc.sync.dma_start(out=wi_sb, in_=wi_d.ap())
            beff_sb = consts.tile([128, 1], F32)
            nc.sync.dma_start(out=beff_sb[0:64, :], in_=beff_d.ap())
            wo1t_sb = consts.tile([128, 32], BF16)
            wo2t4_sb = consts.tile([128, 8], BF16)
            bo1r_sb = consts.tile([128, 1], F32)
            if CFG["gemm"] == "ft":
                from concourse.masks import make_identity
                ident_sb = consts.tile([128, 128], BF16)
                make_identity(nc, ident_sb)
                fsb_pool = ctx.enter_context(tc.tile_pool(name="fsb", bufs=3))

            def emit_head_consts():
                nc.gpsimd.dma_start(out=wo1t_sb[64:128, :], in_=wo1t_d.ap())
                nc.gpsimd.dma_start(out=wo2t4_sb, in_=wo2t4_d.ap())
                nc.gpsimd.dma_start(out=bo1r_sb, in_=bo1r_d.ap())

            xt_sb = xpool.tile([128, N * 8], FP8)
            xk = xt_sb.rearrange("p (n k) -> p k n", k=8)
            xn = xt_sb.rearrange("p (n k) -> p n k", k=8)
            xd = xt_d.ap().rearrange("p (n k) -> p n k", k=8)

            # state+drive: chain g in cols [g*SEC, (g+1)*SEC);
            # f_j on partitions 0:63 at col j*B, s_j on 64:127 at col j*B.
            sf = spool.tile([128, G * SEC], BF16)
            sfg = sf.rearrange("p (g c) -> p g c", g=G)

            # Transfer plan in half-chunk units (m = t/8 = 256 n-cols,
            # 0.25MB, 728ns transfer).  DMA *issue* costs ~1.2us per
            # instruction on one sequencer, so halves alternate between two
            # issue streams (SP HWDGE / Pool SWDGE) to stay transfer-bound.
            # Deadline order: warm sources (m=4g+3), wave j8 (m=4g), wave
            # j16 (m=4g+1), wave j24 (m=4g+2, incl m62), then m63 last.
            if CFG["nh"] == 1:
                XORD = ([(4 * g + 3, 1) for g in range(15)]
                        + [(4 * g, 1) for g in range(G)]
                        + [(4 * g + 1, 1) for g in range(G)]
                        + [(4 * g + 2, 1) for g in range(G)]
                        + [(63, 1)])
            else:
                # full-chunk transfers: r1 chunks (warm srcs) first, then r0
                XORD = ([(4 * g + 2, 2) for g in range(15)]
                        + [(4 * g, 2) for g in range(G)]
                        + [(62, 2)])

            def emit_xdma(m0, nh, eng):
                n0 = m0 * 256
                eng.dma_start(out=xn[:, n0:n0 + nh * 256, :],
                              in_=xd[:, n0:n0 + nh * 256, :])

            evict_rr = [0]
            wp8 = wpack_sb.rearrange("p (k h) -> p k h", k=8)

            def emit_gemm(m, dests):
                """F GEMM on half-chunk m; evict f into each (g, jcol)."""
                n0 = m * 256
                if gemm_mode == "ft":
                    # x stationary (M=128 full array), W moving (N=64):
                    # F^T in PSUM, evict to SBUF bf16, PE-transpose back.
                    ftp = ftp_pool.tile([128, 128], F32, tag="ftp")
                    for half in range(2):
                        nn = n0 + half * 128
                        for kk in range(8):
                            nc.tensor.matmul(
                                ftp[:, half * 64:half * 64 + 64],
                                xk[:, kk, nn:nn + 128], wp8[:, kk],
                                start=(kk == 0), stop=(kk == 7))
                    fsb = fsb_pool.tile([128, 128], BF16, tag="fsb")
                    if evict_rr[0] % 2 == 0:
                        nc.vector.tensor_copy(fsb, ftp)
                    else:
                        nc.scalar.copy(fsb, ftp)
                    evict_rr[0] += 1
                    ftr = ftr_pool.tile([64, 256], BF16, tag="ftr")
                    for half in range(2):
                        nc.tensor.transpose(
                            ftr[:, half * 128:half * 128 + 128],
                            fsb[:, half * 64:half * 64 + 64], ident_sb)
                    fps = ftr
                else:
                    fps = fps_pool.tile([64, 256], F32, tag="fps")
                    if gemm_mode == "dr":
                        for pair in range(4):
                            nc.tensor.matmul(
                                fps, wp4[:, pair],
                                xk[:, 2 * pair:2 * pair + 2, n0:n0 + 256],
                                start=(pair == 0), stop=(pair == 3),
                                perf_mode=DR)
                    else:
                        for kk in range(8):
                            nc.tensor.matmul(
                                fps, wp8[:, kk], xk[:, kk, n0:n0 + 256],
                                start=(kk == 0), stop=(kk == 7))
                for (g, jc) in dests:
                    k = evict_rr[0] % 2
                    evict_rr[0] += 1
                    dst = sfg[0:64, g, jc * B:(jc + 8) * B]
                    if k == 0:
                        nc.scalar.activation(
                            dst, fps, mybir.ActivationFunctionType.Identity,
                            bias=beff_sb[0:64, 0:1], scale=1.0 / WSCALE)
                    else:
                        nc.vector.tensor_scalar(
                            dst, fps, 1.0 / WSCALE, beff_sb[0:64, 0:1],
                            mybir.AluOpType.mult, mybir.AluOpType.add)

            def emit_step(j):
                r = rAB_pool.tile([128, 2 * GW * B], F32, tag="r")
                nc.tensor.matmul(r[64:128, 0:GW * B], wi_sb,
                                 sfg[:, 0:GW, j * B:(j + 1) * B],
                                 start=True, stop=True, tile_position=(0, 64))
                nc.tensor.matmul(r[64:128, GW * B:2 * GW * B], wi_sb,
                                 sfg[:, GW:G, j * B:(j + 1) * B],
                                 start=True, stop=True, tile_position=(0, 64))
                nc.vector.tensor_scalar_max(
                    sfg[64:128, 0:GW, (j + 1) * B:(j + 2) * B],
                    r[64:128, 0:GW * B].rearrange("p (g c) -> p g c", g=GW),
                    0.0)
                nc.scalar.activation(
                    sfg[64:128, GW:G, (j + 1) * B:(j + 2) * B],
                    r[64:128, GW * B:2 * GW * B].rearrange(
                        "p (g c) -> p g c", g=GW), RELU)

            def emit_mm1(rr, q, hps, lo, hi):
                """Head layer-1 matmuls for super (rr, q), cols [lo, hi)."""
                c0 = (9 + 16 * rr) * B
                for i in range(4):
                    g = 4 * q + i
                    nc.tensor.matmul(
                        hps[32 * i:32 * (i + 1), lo:hi], wo1t_sb[64:128, :],
                        sfg[64:128, g, c0 + lo:c0 + hi],
                        start=True, stop=True, tile_position=(64, 32 * i))

            def emit_head_rest(q, hps, osb, reng):
                """relu -> W_o2 matmul -> copy into the merged out tile."""
                hs = hspool.tile([128, 512], BF16, tag="hs")
                if reng == 0:
                    nc.vector.tensor_scalar(
                        hs, hps, bo1r_sb[:, 0:1], 0.0,
                        mybir.AluOpType.add, mybir.AluOpType.max)
                else:
                    nc.scalar.activation(hs, hps, RELU, bias=bo1r_sb[:, 0:1])
                ops = ops_pool.tile([8, 512], F32, tag="ops")
                nc.tensor.matmul(ops, wo2t4_sb, hs, start=True, stop=True)
                dst = osb[:, q * 512:(q + 1) * 512]
                if reng == 0:
                    nc.vector.tensor_copy(dst, ops)
                else:
                    nc.scalar.copy(dst, ops)

            def emit_super(rr, q, osb, reng):
                hps = hps_pool.tile([128, 512], F32, tag="hps")
                emit_mm1(rr, q, hps, 0, 512)
                emit_head_rest(q, hps, osb, reng)

            # JIT wave-GEMM schedule: wave consumed at jw gets its 16 GEMMs
            # spread 2-per-j across the preceding 8-step block (arrival-
            # paced); head supers for r0 land mid-run, r1 at the tail.
            gemm_at = {j: [] for j in range(SPC + 1)}
            for g in range(G):
                gemm_at[0 + g // 2].append((4 * g, [(g, 8)]))
                gemm_at[8 + g // 2].append((4 * g + 1, [(g, 16)]))
                gemm_at[16 + g // 2].append((4 * g + 2, [(g, 24)]))
            gemm_at[31].append((63, [(15, 32)]))

            for rep in range(repeats):
                nc.vector.memset(sfg[64:128, 0:G, 0:B], 0.0)   # s_0 = 0
                nc.vector.memset(sf[0:64, 0:WARM * B], 0.0)    # chain0 warm f
                for i, (m0, nh) in enumerate(XORD):
                    emit_xdma(m0, nh, nc.sync if i % 2 == 0 else nc.gpsimd)
                    if rep == 0 and i == len(XORD) // 2:
                        emit_head_consts()
                # warm GEMMs double-evict: chain g's r3 block + g+1's warm
                for g in range(15):
                    emit_gemm(4 * g + 3, [(g, 32), (g + 1, 0)])
                osb0 = opool.tile([8, 2048], F32, tag="osb")
                osb1 = opool.tile([8, 2048], F32, tag="osb")
                tail_hps = {}
                for j in range(SPC):
                    emit_step(j)
                    for (m, dests) in gemm_at[j]:
                        emit_gemm(m, dests)
                    if j in (25, 27, 29, 31):
                        emit_super(0, (j - 25) // 2, osb0, (j - 25) // 2 % 2)
                    if j == 31:
                        nc.sync.dma_start(
                            out=out_d.ap()[:, 0:2048], in_=osb0)
                    if j == 33:
                        # first-half head mm1s for tail supers 0,1 (their s
                        # cols 9..25 of r1 exist after step 32's relu)
                        for q in (0, 1):
                            hps = hps_pool.tile([128, 512], F32, tag="hps")
                            emit_mm1(1, q, hps, 0, 256)
                            tail_hps[q] = hps
                # tail: finish supers 0,1 then 2,3, stage-major
                for q in (0, 1):
                    emit_mm1(1, q, tail_hps[q], 256, 512)
                for q in (0, 1):
                    emit_head_rest(q, tail_hps[q], osb1, q % 2)
                for q in (2, 3):
                    hps = hps_pool.tile([128, 512], F32, tag="hps")
                    emit_mm1(1, q, hps, 0, 512)
                    tail_hps[q] = hps
                for q in (2, 3):
                    emit_head_rest(q, tail_hps[q], osb1, q % 2)
                nc.sync.dma_start(out=out_d.ap()[:, 2048:4096], in_=osb1)

    return _split_multiwaits(nc)


_NC_CACHE = None


def _get_nc():
    global _NC_CACHE
    if _NC_CACHE is None:
        _NC_CACHE = build_decoder_nc()
    return _NC_CACHE


def make_in_maps(inputs):
    x = np.asarray(inputs["x"], np.float32)
    W_in = np.asarray(inputs["W_in"], np.float32)
    b_in = np.asarray(inputs["b_in"], np.float32)
    W_rec = np.asarray(inputs["W_rec"], np.float32)
    b_rec = np.asarray(inputs["b_rec"], np.float32)
    W_o1 = np.asarray(inputs["W_o1"], np.float32)
    b_o1 = np.asarray(inputs["b_o1"], np.float32)
    W_o2 = np.asarray(inputs["W_o2"], np.float32)

    W_eff = (W_rec @ W_in).astype(np.float32)            # [64, 1024]
    b_eff = (W_rec @ b_in + b_rec).astype(np.float32)    # [64]

    f8 = ml_dtypes.float8_e4m3
    bf = ml_dtypes.bfloat16
    wq = (WSCALE * W_eff).astype(f8)
    # wpack[p, pair*128 + dd*64 + h] = wq[h, (2*pair+dd)*128 + p]
    wpack = np.ascontiguousarray(
        wq.reshape(64, 4, 2, 128).transpose(3, 1, 2, 0)).reshape(128, 512)
    wi = np.zeros((128, 64), np.float32)
    wi[0:64] = np.eye(64)          # f is descaled at eviction time
    wi[64:128] = W_rec.T
    wo2t4 = np.zeros((128, 8), np.float32)
    for i in range(4):
        wo2t4[32 * i:32 * (i + 1), 2 * i:2 * (i + 1)] = W_o2.T

    shared = {
        "wpack": wpack,
        "wi": wi.astype(bf),
        "wo1t": np.ascontiguousarray(W_o1.T).astype(bf),
        "wo2t4": wo2t4.astype(bf),
        "beff": np.ascontiguousarray(b_eff[:, None]),
        "bo1r": np.ascontiguousarray(np.tile(b_o1, 4)[:, None]),
    }
    in_maps = []
    for cid in range(NCORES):
        xs = x[cid * B:(cid + 1) * B]                    # [B, T, S]
        xt = xs.reshape(B, T, 8, 128).transpose(3, 1, 0, 2)  # [p, t, b, k]
        xt = np.ascontiguousarray(xt).reshape(128, N * 8).astype(f8)
        m = dict(shared)
        m["xt"] = xt
        in_maps.append(m)
    return in_maps


def kernel(**inputs):
    b_o2 = np.asarray(inputs["b_o2"], np.float32)
    in_maps = make_in_maps(inputs)
    res = run_bass_kernel_spmd(_get_nc(), in_maps,
                               core_ids=list(range(NCORES)))

    out = np.empty((BS, T, 2), np.float32)
    for cid in range(NCORES):
        o4 = np.asarray(res.results[cid]["out4"])        # [8, 8*512]
        for rr in range(2):
            for q in range(4):
                si = rr * 4 + q
                blk = o4[:, si * 512:(si + 1) * 512].reshape(4, 2, 16, B)
                for i in range(4):
                    g = 4 * q + i
                    t0 = 32 * g + 16 * rr
                    out[cid * B:(cid + 1) * B, t0:t0 + 16, :] = \
                        blk[i].transpose(2, 1, 0)
    out += b_o2[None, None, :]
    return out


# revision 19
# speedup vs baseline: 2.2040x; 2.2040x over previous
"""Trainium2 Bass kernel for nn_Decoder (input proj -> relu RNN -> 2-layer head).

Strategy (8 NeuronCores, pure batch data-parallelism, B=32 rows/core):
  - Fold the input projection into the recurrence drive on the host:
        f_t = W_eff @ x_t^T,  W_eff = 64 * (W_rec @ W_in)  (fp8 e4m3, x in fp8)
        s_{t+1} = relu(W_rec s_t + f_t/64 + b_eff)
  - x is host-pretransposed to [p, n, k] (s = k*128+p, n = t*B+b) and
    host-cast to fp8e4, so NO on-chip transposes and HALF the HBM bytes of
    bf16.  All 16.8MB/core stay RESIDENT in SBUF.
  - F GEMM runs as fp8 DoubleRow matmuls (0.5 cyc/col, K=256 per
    instruction), accumulating into PSUM partitions 0:63; a gpsimd
    (Pool-engine) tensor_scalar evicts f = PSUM/64 + b_eff into the [f; s]
    state buffer (f on partitions 0:63, s on 64:127) as bf16.
  - The 512-step recurrence runs as 16 CONCURRENT chains of 32 real steps
    (+8 warm steps from zero; ||W_rec||~0.34 so 0.34^8 ~ 2e-4).  Chains are
    fused into 2 groups of 8: one [128Kx64M, 256N] matmul per group-step
    (lhsT = [[I/64],[W_rec^T]], tile_position (0,64)), relu on DVE for
    group A and on ScalarE for group B -> ~850ns per fused step for all
    16 chains.
  - x streams in 64 half-chunks (256 n-cols, 2KB runs) ordered by consume
    deadline: warm sources (= r3 blocks of the previous chain) first, then
    wave r0/r1/r2, chain-15's final half last.  GEMM emission is staged
    between step blocks so the in-order PE queue never deadlocks.
  - Head: 4 chunks' relu(W_o1 s + b_o1) stacked on partitions via
    tile_position (64, 32i) -> ONE [128,512] relu -> one [128K, 8M] W_o2
    matmul -> PSUM->SBUF copy -> DMA out.  b_o2 is added on the host.
"""

import sys
import json
import numpy as np

for _p in ("/opt/trn_rl_repo",):
    if _p not in sys.path:
        sys.path.insert(0, _p)

import ml_dtypes
import concourse.bass as bass
import concourse.mybir as mybir
import concourse.tile as tile
from concourse.bass_utils import run_bass_kernel_spmd
from contextlib import ExitStack

BS, T, S, H = 256, 512, 1024, 64
NCORES = 8
B = BS // NCORES          # 32 batch rows per core
N = T * B                 # 16384 columns (n = t*B + b)
G = 16                    # chains
GW = 8                    # chains per fused group
RSTEP = T // G            # 32 real steps per chain
WARM = 8
SPC = WARM + RSTEP        # 40 steps per chain
SEC = (SPC + 1) * B       # 1312 cols per chain section
WSCALE = 64.0

F32 = mybir.dt.float32
BF16 = mybir.dt.bfloat16
FP8 = mybir.dt.float8e4
DR = mybir.MatmulPerfMode.DoubleRow
RELU = mybir.ActivationFunctionType.Relu

# Build-time config (A/B testing): GEMM mode and x-DMA granularity.
import os as _os
CFG = {"gemm": _os.environ.get("GEMM_MODE", "ft"),
       "nh": int(_os.environ.get("DMA_NH", "2"))}


def _split_multiwaits(nc, max_waits=1):
    """walrus in this container rejects >1 sem-wait on one instruction (the
    Tile end-of-kernel drain carries several).  Split extras into chained
    same-engine NoOps, then pin the serialized bytes on the nc object."""
    j = json.loads(nc.to_json_bytes())
    for f in j["functions"]:
        for bb in f["blocks"]:
            newinsts = []
            for inst in bb["instructions"]:
                si = inst.get("sync_info")
                waits = (si or {}).get("on_wait") or []
                if len(waits) > max_waits:
                    for k, w in enumerate(waits[max_waits:]):
                        newinsts.append({
                            "debug": inst.get("debug"),
                            "engine": inst["engine"],
                            "ins": [], "outs": [],
                            "name": f'{inst["name"]}-xw{k}',
                            "opcode": "NoOp",
                            "sync_info": {"on_update": [], "on_wait": [w]},
                        })
                    si["on_wait"] = waits[:max_waits]
                newinsts.append(inst)
            bb["instructions"] = newinsts
    b = json.dumps(j).encode()
    nc.to_json_bytes = lambda: b
    return nc


def build_decoder_nc(repeats=1):
    nc = bass.Bass("TRN2", target_bir_lowering=False, debug=False)

    # transfer-major layout: block m (= t/8, 256 n-cols) is one fully
    # contiguous 256KB DRAM region [p, n-in-block, k]
    xt_d = nc.dram_tensor("xt", [64, 128, 2048], FP8, kind="ExternalInput")
    wpack_d = nc.dram_tensor("wpack", [128, 512], FP8, kind="ExternalInput")
    wi_d = nc.dram_tensor("wi", [128, H], BF16, kind="ExternalInput")
    wo1t_d = nc.dram_tensor("wo1t", [H, 32], BF16, kind="ExternalInput")
    wo2t4_d = nc.dram_tensor("wo2t4", [128, 8], BF16, kind="ExternalInput")
    beff_d = nc.dram_tensor("beff", [H, 1], F32, kind="ExternalInput")
    bo1r_d = nc.dram_tensor("bo1r", [128, 1], F32, kind="ExternalInput")
    out_d = nc.dram_tensor("out4", [8, 8 * 512], F32, kind="ExternalOutput")

    with tile.TileContext(nc) as tc:
        with ExitStack() as ctx:
            consts = ctx.enter_context(tc.tile_pool(name="consts", bufs=1))
            xpool = ctx.enter_context(tc.tile_pool(name="xt", bufs=1))
            spool = ctx.enter_context(tc.tile_pool(name="sf", bufs=1))
            hspool = ctx.enter_context(tc.tile_pool(name="hs", bufs=2))
            opool = ctx.enter_context(tc.tile_pool(name="osb", bufs=2))
            gemm_mode = CFG["gemm"]
            if gemm_mode == "ft":
                ftp_pool = ctx.enter_context(
                    tc.tile_pool(name="ftp", bufs=2, space="PSUM"))
                ftr_pool = ctx.enter_context(
                    tc.tile_pool(name="ftr", bufs=2, space="PSUM"))
            else:
                fps_pool = ctx.enter_context(
                    tc.tile_pool(name="fps", bufs=3, space="PSUM"))
            rAB_pool = ctx.enter_context(
                tc.tile_pool(name="rAB", bufs=1, space="PSUM"))
            hps_pool = ctx.enter_context(
                tc.tile_pool(name="hps", bufs=2, space="PSUM"))
            ops_pool = ctx.enter_context(
                tc.tile_pool(name="ops", bufs=1, space="PSUM"))

            # --- constants (loaded once); GEMM/step consts go first, the
            # head consts are issued behind the x stream (needed ~j24) ---
            wpack_sb = consts.tile([128, 512], FP8)
            nc.sync.dma_start(out=wpack_sb, in_=wpack_d.ap())
            wp4 = wpack_sb.rearrange("p (a d h) -> p a d h", a=4, d=2)
            wi_sb = consts.tile([128, H], BF16)
            nc.sync.dma_start(out=wi_sb, in_=wi_d.ap())
            beff_sb = consts.tile([128, 1], F32)
            nc.sync.dma_start(out=beff_sb[0:64, :], in_=beff_d.ap())
            wo1t_sb = consts.tile([128, 32], BF16)
            wo2t4_sb = consts.tile([128, 8], BF16)
            bo1r_sb = consts.tile([128, 1], F32)
            if CFG["gemm"] == "ft":
                from concourse.masks import make_identity
                ident_sb = consts.tile([128, 128], BF16)
                make_identity(nc, ident_sb)
                fsb_pool = ctx.enter_context(tc.tile_pool(name="fsb", bufs=3))

            def emit_head_consts():
                nc.gpsimd.dma_start(out=wo1t_sb[64:128, :], in_=wo1t_d.ap())
                nc.gpsimd.dma_start(out=wo2t4_sb, in_=wo2t4_d.ap())
                nc.gpsimd.dma_start(out=bo1r_sb, in_=bo1r_d.ap())

            xt_sb = xpool.tile([128, N * 8], FP8)
            xk = xt_sb.rearrange("p (n k) -> p k n", k=8)
            xn = xt_sb.rearrange("p (n k) -> p n k", k=8)
            xd = xt_d.ap()

            # state+drive: chain g in cols [g*SEC, (g+1)*SEC);
            # f_j on partitions 0:63 at col j*B, s_j on 64:127 at col j*B.
            sf = spool.tile([128, G * SEC], BF16)
            sfg = sf.rearrange("p (g c) -> p g c", g=G)

            # Transfer plan in half-chunk units (m = t/8 = 256 n-cols,
            # 0.25MB, 728ns transfer).  DMA *issue* costs ~1.2us per
            # instruction on one sequencer, so halves alternate between two
            # issue streams (SP HWDGE / Pool SWDGE) to stay transfer-bound.
            # Deadline order: warm sources (m=4g+3), wave j8 (m=4g), wave
            # j16 (m=4g+1), wave j24 (m=4g+2, incl m62), then m63 last.
            if CFG["nh"] == 1:
                XORD = ([(4 * g + 3, 1) for g in range(15)]
                        + [(4 * g, 1) for g in range(G)]
                        + [(4 * g + 1, 1) for g in range(G)]
                        + [(4 * g + 2, 1) for g in range(G)]
                        + [(63, 1)])
            else:
                # full-chunk transfers: r1 chunks (warm srcs) first, then r0
                XORD = ([(4 * g + 2, 2) for g in range(15)]
                        + [(4 * g, 2) for g in range(G)]
                        + [(62, 2)])

            def emit_xdma(m0, nh, eng):
                n0 = m0 * 256
                dst = xn[:, n0:n0 + nh * 256, :].rearrange(
                    "p (m n) k -> p m n k", m=nh)
                src_ = xd[m0:m0 + nh].rearrange(
                    "m p (n k) -> p m n k", k=8)
                eng.dma_start(out=dst, in_=src_)

            evict_rr = [0]
            wp8 = wpack_sb.rearrange("p (k h) -> p k h", k=8)

            def emit_gemm(m, dests):
                """F GEMM on half-chunk m; evict f into each (g, jcol)."""
                n0 = m * 256
                if gemm_mode == "ft":
                    # x stationary (M=128 full array), W moving (N=64):
                    # F^T in PSUM, evict to SBUF bf16, PE-transpose back.
                    ftp = ftp_pool.tile([128, 128], F32, tag="ftp")
                    for half in range(2):
                        nn = n0 + half * 128
                        for kk in range(8):
                            nc.tensor.matmul(
                                ftp[:, half * 64:half * 64 + 64],
                                xk[:, kk, nn:nn + 128], wp8[:, kk],
                                start=(kk == 0), stop=(kk == 7))
                    fsb = fsb_pool.tile([128, 128], BF16, tag="fsb")
                    if evict_rr[0] % 2 == 0:
                        nc.vector.tensor_copy(fsb, ftp)
                    else:
                        nc.scalar.copy(fsb, ftp)
                    evict_rr[0] += 1
                    ftr = ftr_pool.tile([64, 256], BF16, tag="ftr")
                    for half in range(2):
                        nc.tensor.transpose(
                            ftr[:, half * 128:half * 128 + 128],
                            fsb[:, half * 64:half * 64 + 64], ident_sb)
                    fps = ftr
                else:
                    fps = fps_pool.tile([64, 256], F32, tag="fps")
                    if gemm_mode == "dr":
                        for pair in range(4):
                            nc.tensor.matmul(
                                fps, wp4[:, pair],
                                xk[:, 2 * pair:2 * pair + 2, n0:n0 + 256],
                                start=(pair == 0), stop=(pair == 3),
                                perf_mode=DR)
                    else:
                        for kk in range(8):
                            nc.tensor.matmul(
                                fps, wp8[:, kk], xk[:, kk, n0:n0 + 256],
                                start=(kk == 0), stop=(kk == 7))
                for (g, jc) in dests:
                    k = evict_rr[0] % 2
                    evict_rr[0] += 1
                    dst = sfg[0:64, g, jc * B:(jc + 8) * B]
                    if k == 0:
                        nc.scalar.activation(
                            dst, fps, mybir.ActivationFunctionType.Identity,
                            bias=beff_sb[0:64, 0:1], scale=1.0 / WSCALE)
                    else:
                        nc.vector.tensor_scalar(
                            dst, fps, 1.0 / WSCALE, beff_sb[0:64, 0:1],
                            mybir.AluOpType.mult, mybir.AluOpType.add)

            def emit_step(j):
                r = rAB_pool.tile([128, 2 * GW * B], F32, tag="r")
                nc.tensor.matmul(r[64:128, 0:GW * B], wi_sb,
                                 sfg[:, 0:GW, j * B:(j + 1) * B],
                                 start=True, stop=True, tile_position=(0, 64))
                nc.tensor.matmul(r[64:128, GW * B:2 * GW * B], wi_sb,
                                 sfg[:, GW:G, j * B:(j + 1) * B],
                                 start=True, stop=True, tile_position=(0, 64))
                nc.vector.tensor_scalar_max(
                    sfg[64:128, 0:GW, (j + 1) * B:(j + 2) * B],
                    r[64:128, 0:GW * B].rearrange("p (g c) -> p g c", g=GW),
                    0.0)
                nc.scalar.activation(
                    sfg[64:128, GW:G, (j + 1) * B:(j + 2) * B],
                    r[64:128, GW * B:2 * GW * B].rearrange(
                        "p (g c) -> p g c", g=GW), RELU)

            def emit_mm1(rr, q, hps, lo, hi):
                """Head layer-1 matmuls for super (rr, q), cols [lo, hi)."""
                c0 = (9 + 16 * rr) * B
                for i in range(4):
                    g = 4 * q + i
                    nc.tensor.matmul(
                        hps[32 * i:32 * (i + 1), lo:hi], wo1t_sb[64:128, :],
                        sfg[64:128, g, c0 + lo:c0 + hi],
                        start=True, stop=True, tile_position=(64, 32 * i))

            def emit_head_rest(q, hps, osb, reng):
                """relu -> W_o2 matmul -> copy into the merged out tile."""
                hs = hspool.tile([128, 512], BF16, tag="hs")
                if reng == 0:
                    nc.vector.tensor_scalar(
                        hs, hps, bo1r_sb[:, 0:1], 0.0,
                        mybir.AluOpType.add, mybir.AluOpType.max)
                else:
                    nc.scalar.activation(hs, hps, RELU, bias=bo1r_sb[:, 0:1])
                ops = ops_pool.tile([8, 512], F32, tag="ops")
                nc.tensor.matmul(ops, wo2t4_sb, hs, start=True, stop=True)
                dst = osb[:, q * 512:(q + 1) * 512]
                if reng == 0:
                    nc.vector.tensor_copy(dst, ops)
                else:
                    nc.scalar.copy(dst, ops)

            def emit_super(rr, q, osb, reng):
                hps = hps_pool.tile([128, 512], F32, tag="hps")
                emit_mm1(rr, q, hps, 0, 512)
                emit_head_rest(q, hps, osb, reng)

            # JIT wave-GEMM schedule: wave consumed at jw gets its 16 GEMMs
            # spread 2-per-j across the preceding 8-step block (arrival-
            # paced); head supers for r0 land mid-run, r1 at the tail.
            gemm_at = {j: [] for j in range(SPC + 1)}
            for g in range(G):
                gemm_at[0 + g // 2].append((4 * g, [(g, 8)]))
                gemm_at[8 + g // 2].append((4 * g + 1, [(g, 16)]))
                gemm_at[16 + g // 2].append((4 * g + 2, [(g, 24)]))
            gemm_at[31].append((63, [(15, 32)]))

            for rep in range(repeats):
                nc.vector.memset(sfg[64:128, 0:G, 0:B], 0.0)   # s_0 = 0
                nc.vector.memset(sf[0:64, 0:WARM * B], 0.0)    # chain0 warm f
                for i, (m0, nh) in enumerate(XORD):
                    emit_xdma(m0, nh, nc.sync if i % 2 == 0 else nc.gpsimd)
                    if rep == 0 and i == len(XORD) // 2:
                        emit_head_consts()
                # warm GEMMs double-evict: chain g's r3 block + g+1's warm
                for g in range(15):
                    emit_gemm(4 * g + 3, [(g, 32), (g + 1, 0)])
                osb0 = opool.tile([8, 2048], F32, tag="osb")
                osb1 = opool.tile([8, 2048], F32, tag="osb")
                tail_hps = {}
                for j in range(SPC):
                    emit_step(j)
                    for (m, dests) in gemm_at[j]:
                        emit_gemm(m, dests)
                    if j in (25, 27, 29, 31):
                        emit_super(0, (j - 25) // 2, osb0, (j - 25) // 2 % 2)
                    if j == 31:
                        nc.sync.dma_start(
                            out=out_d.ap()[:, 0:2048], in_=osb0)
                    if j == 33:
                        # first-half head mm1s for tail supers 0,1 (their s
                        # cols 9..25 of r1 exist after step 32's relu)
                        for q in (0, 1):
                            hps = hps_pool.tile([128, 512], F32, tag="hps")
                            emit_mm1(1, q, hps, 0, 256)
                            tail_hps[q] = hps
                # tail: finish supers 0,1 then 2,3, stage-major
                for q in (0, 1):
                    emit_mm1(1, q, tail_hps[q], 256, 512)
                for q in (0, 1):
                    emit_head_rest(q, tail_hps[q], osb1, q % 2)
                for q in (2, 3):
                    hps = hps_pool.tile([128, 512], F32, tag="hps")
                    emit_mm1(1, q, hps, 0, 512)
                    tail_hps[q] = hps
                for q in (2, 3):
                    emit_head_rest(q, tail_hps[q], osb1, q % 2)
                nc.sync.dma_start(out=out_d.ap()[:, 2048:4096], in_=osb1)

    return _split_multiwaits(nc)


_NC_CACHE = None


def _get_nc():
    global _NC_CACHE
    if _NC_CACHE is None:
        _NC_CACHE = build_decoder_nc()
    return _NC_CACHE


def make_in_maps(inputs):
    x = np.asarray(inputs["x"], np.float32)
    W_in = np.asarray(inputs["W_in"], np.float32)
    b_in = np.asarray(inputs["b_in"], np.float32)
    W_rec = np.asarray(inputs["W_rec"], np.float32)
    b_rec = np.asarray(inputs["b_rec"], np.float32)
    W_o1 = np.asarray(inputs["W_o1"], np.float32)
    b_o1 = np.asarray(inputs["b_o1"], np.float32)
    W_o2 = np.asarray(inputs["W_o2"], np.float32)

    W_eff = (W_rec @ W_in).astype(np.float32)            # [64, 1024]
    b_eff = (W_rec @ b_in + b_rec).astype(np.float32)    # [64]

    f8 = ml_dtypes.float8_e4m3
    bf = ml_dtypes.bfloat16
    wq = (WSCALE * W_eff).astype(f8)
    # wpack[p, pair*128 + dd*64 + h] = wq[h, (2*pair+dd)*128 + p]
    wpack = np.ascontiguousarray(
        wq.reshape(64, 4, 2, 128).transpose(3, 1, 2, 0)).reshape(128, 512)
    wi = np.zeros((128, 64), np.float32)
    wi[0:64] = np.eye(64)          # f is descaled at eviction time
    wi[64:128] = W_rec.T
    wo2t4 = np.zeros((128, 8), np.float32)
    for i in range(4):
        wo2t4[32 * i:32 * (i + 1), 2 * i:2 * (i + 1)] = W_o2.T

    shared = {
        "wpack": wpack,
        "wi": wi.astype(bf),
        "wo1t": np.ascontiguousarray(W_o1.T).astype(bf),
        "wo2t4": wo2t4.astype(bf),
        "beff": np.ascontiguousarray(b_eff[:, None]),
        "bo1r": np.ascontiguousarray(np.tile(b_o1, 4)[:, None]),
    }
    in_maps = []
    for cid in range(NCORES):
        xs = x[cid * B:(cid + 1) * B]                    # [B, T, S]
        xt = xs.reshape(B, T, 8, 128).transpose(3, 1, 0, 2)  # [p, t, b, k]
        xt = np.ascontiguousarray(xt).reshape(128, N * 8).astype(f8)
        # transfer-major: [m, p, 2048] with block m contiguous in DRAM
        xt = np.ascontiguousarray(xt.reshape(128, 64, 2048).transpose(1, 0, 2))
        m = dict(shared)
        m["xt"] = xt
        in_maps.append(m)
    return in_maps


def kernel(**inputs):
    b_o2 = np.asarray(inputs["b_o2"], np.float32)
    in_maps = make_in_maps(inputs)
    res = run_bass_kernel_spmd(_get_nc(), in_maps,
                               core_ids=list(range(NCORES)))

    out = np.empty((BS, T, 2), np.float32)
    for cid in range(NCORES):
        o4 = np.asarray(res.results[cid]["out4"])        # [8, 8*512]
        for rr in range(2):
            for q in range(4):
                si = rr * 4 + q
                blk = o4[:, si * 512:(si + 1) * 512].reshape(4, 2, 16, B)
                for i in range(4):
                    g = 4 * q + i
                    t0 = 32 * g + 16 * rr
                    out[cid * B:(cid + 1) * B, t0:t0 + 16, :] = \
                        blk[i].transpose(2, 1, 0)
    out += b_o2[None, None, :]
    return out
